# revision 1
# baseline (speedup 1.0000x reference)
"""MoE-routed conv kernel (Channel_Embedding ablation) for 8 trn2 NeuronCores.

Math (see reference):
  gates  = top2-renormalized softmax( x[:, :, -6:-1].reshape(B, D*5) @ w_gate )
  h      = tanh(conv1d(x, conv1_w, VALID) + conv1_b)            # [B, OC, L-2]
  out    = conv1d(h, conv2_w, 1x1) + conv2_b                    # [B, OC*E, L-2]
  y[b,oc,t] = sum_e gates[b,e] * out[b, oc*E+e, t]

Key algebraic fold: the expert combine commutes with the 1x1 conv, so per
batch element
  W_eff[b][oc, ic] = sum_e gates[b,e] * conv2_w[oc*E+e, ic, 0]
  b_eff[b][oc]     = sum_e gates[b,e] * conv2_b[oc*E+e]
  y[b] = W_eff[b] @ h[b] + b_eff[b]
collapsing the 256-channel conv2 into a 32x32 matmul and making the kernel
memory-bound: per core (4 batch elements) ~4.2MB in + ~2.1MB out.

Sharding: data-parallel over batch B=32 across 8 cores (4 each); weights
replicated. Within a core, batches {p, p+2} form pair p; each conv matmul
stacks a pair along the contraction dim (K=128) with block-diagonal weights
(fp32(r) matmuls pair PE column groups, so M=64 outputs must land at PSUM
partition 0 — hence per-pair [64, *] tiles). conv/combine matmuls run in
float32r (fp32 bits, ~4x faster PE), giving ~2e-4 rel err; gating and
W_eff run in strict fp32.

Schedule: x is loaded into a persistent SBUF image in column chunks, TAIL
chunk first so the gating window (last 5 columns) is available immediately;
gating -> W_eff -> combine-weight scatter completes during the bulk x load.
Outputs accumulate in persistent SBUF and are stored per batch half-length.
DMA instruction count is kept low (per-DMA DGE overhead is ~0.6-1us).
"""

from contextlib import ExitStack

import numpy as np

import concourse.bacc as bacc
import concourse.mybir as mybir
import concourse.tile as tile
from concourse import bass_utils

B, D, L = 32, 64, 4096
E, TOPK, OC = 8, 2, 32
LP = L - 2  # 4094 valid conv outputs
NCORES = 8
NB = B // NCORES  # batch elements per core
TS = 512  # position tile (one PSUM bank of fp32)
NT = (LP + TS - 1) // TS

# fp32-bit matmul dtype that runs ~4x faster on the PE than strict float32.
FAST_DT = mybir.dt.float32r

# f32 constants image [128, NCONST_F]: gating weights (duplicated in both
# partition halves) + conv1 bias.
C_WG = 0  # rows 0:64 AND 64:128, [*, 40], col = t*8 + e
C_B1P = C_WG + 5 * E  # rows 0:64, [64, 1] conv1 bias (tiled x2)
NCONST_F = C_B1P + 1
NW1T = 3 * 2 * OC  # fp32r image: block-diag conv1 weights [128, 192]
# fp32r conv2 image [8, 1056]: c2w[e, ic*32+oc], c2b[e, oc]
C2_W, C2_B, NC2 = 0, OC * OC, OC * OC + OC

_CACHE: dict = {}


def _softmax_top2(nc, sm, lg, f32, AX, OP, AF, q):
    """Per-half gating: lg [2, E] logits (PSUM) -> gates [2, E] in SBUF.

    gates = (e >= m2) * e / (m1 + m2 + 1e-6 * sum(e)), e = exp(logits) —
    identical to softmax -> top2 -> vk/(sum vk + 1e-6) in exact arithmetic.
    """
    e_sb = sm.tile([2, E], f32, name=f"e_sb{q}")
    nc.scalar.activation(e_sb[:], lg[:], AF.Exp)
    m1 = sm.tile([2, 1], f32, name=f"m1_{q}")
    nc.vector.reduce_max(m1[:], e_sb[:], axis=AX.X)
    lt = sm.tile([2, E], f32, name=f"lt{q}")
    nc.vector.tensor_scalar(lt[:], e_sb[:], m1[:], None, op0=OP.is_lt)
    emsk = sm.tile([2, E], f32, name=f"emsk{q}")
    nc.vector.tensor_mul(emsk[:], lt[:], e_sb[:])  # e with the max zeroed
    m2 = sm.tile([2, 1], f32, name=f"m2_{q}")
    nc.vector.reduce_max(m2[:], emsk[:], axis=AX.X)
    den3 = sm.tile([2, 1], f32, name=f"den3{q}")
    nc.vector.tensor_add(den3[:], m1[:], m2[:])
    rcp = sm.tile([2, 1], f32, name=f"rcp{q}")
    nc.vector.reciprocal(rcp[:], den3[:])
    ge = sm.tile([2, E], f32, name=f"ge{q}")
    nc.vector.tensor_scalar(ge[:], e_sb[:], m2[:], None, op0=OP.is_ge)
    gnum = sm.tile([2, E], f32, name=f"gnum{q}")
    nc.vector.tensor_mul(gnum[:], ge[:], e_sb[:])
    gpad = sm.tile([32, 32], f32, name=f"gpad{q}")
    nc.vector.memset(gpad[:], 0.0)
    nc.vector.tensor_scalar(gpad[0:2, 0:E], gnum[:], rcp[:], None, op0=OP.mult)
    gtr = sm.tile([32, 32], f32, name=f"gtr{q}")
    nc.vector.transpose(gtr[:], gpad[:])  # 32x32 block transpose
    return gtr  # gtr[0:E, 0:2] = gates.T for batches {2q, 2q+1}


def _emit(ctx, tc, nc, x_d, cr_d, cf_d, c2r_d, y_d):
    f32 = mybir.dt.float32
    AF = mybir.ActivationFunctionType
    AX = mybir.AxisListType
    OP = mybir.AluOpType

    const = ctx.enter_context(tc.tile_pool(name="const", bufs=1))
    sm = ctx.enter_context(tc.tile_pool(name="sm", bufs=1))
    hsb = ctx.enter_context(tc.tile_pool(name="hsb", bufs=16))
    psum_h = ctx.enter_context(tc.tile_pool(name="ph", bufs=3, space="PSUM"))
    psum_o = ctx.enter_context(tc.tile_pool(name="po", bufs=3, space="PSUM"))
    psum_s = ctx.enter_context(tc.tile_pool(name="ps", bufs=2, space="PSUM"))
    dram = ctx.enter_context(tc.tile_pool(name="dram", bufs=1, space="DRAM"))

    # ---- x image: xf[64q + d, 4096p + c] = x[2q + p, d, c]
    # (batch b = 2q + p; conv pair p stacks the two q halves on K=128).
    # DMA issue order = DMA device queue order: x TAIL chunks first (the
    # gating window is x[:, :, L-6:L-1]), then the small constants, then
    # the bulk x chunks.
    xf = const.tile([2 * D, 2 * L], FAST_DT)
    xv = x_d.ap().rearrange("(q p) d c -> q d p c", q=2)

    def load_chunk(a0, a1):
        for q in range(2):
            nc.sync.dma_start(
                xf[D * q : D * q + D, :].rearrange("d (p c) -> d p c", p=2)[
                    :, :, a0:a1
                ],
                xv[q : q + 1, :, :, a0:a1].bitcast(FAST_DT),
            )

    load_chunk(3584, 4096)  # tail (gating window + last conv tile)

    cf = const.tile([128, NCONST_F], f32)
    nc.sync.dma_start(cf[:], cf_d.ap(), max_dma_last_dim=NCONST_F)
    cr = const.tile([2 * D, NW1T], FAST_DT)
    nc.sync.dma_start(cr[:], cr_d.ap().bitcast(FAST_DT), max_dma_last_dim=NW1T)
    c2r = const.tile([E, NC2], FAST_DT)
    nc.sync.dma_start(c2r[:], c2r_d.ap().bitcast(FAST_DT), max_dma_last_dim=NC2)
    w1t = cr  # lhsT tap k = w1t[:, 64k:64k+64] = [[w1T_k, 0], [0, w1T_k]]
    c2w = c2r[0:E, C2_W : C2_W + OC * OC]
    c2b = c2r[0:E, C2_B : C2_B + OC]
    b1p = cf[0 : 2 * OC, C_B1P : C_B1P + 1]

    for i in range(7):  # bulk x
        load_chunk(512 * i, 512 * (i + 1))

    # ---- ACT table warmup (exp/tanh share one table set; load it early)
    warm = sm.tile([1, 8], f32)
    nc.vector.memset(warm[:], 0.0)
    warm2 = sm.tile([1, 8], f32)
    nc.scalar.activation(warm2[:], warm[:], AF.Exp)

    # ---- PE warmup: dummy matmuls on a memset tile (no data deps) so the
    # PE clock ramp happens during the x load window, not on the first real
    # conv/W_eff matmuls.
    wsrc = sm.tile([128, 64], f32)
    nc.vector.memset(wsrc[:], 0.0)
    wup = psum_s.tile([2 * OC, 64], f32, tag="s")
    for _ in range(10):
        nc.tensor.matmul(wup[:], wsrc[:], wsrc[:], start=True, stop=True)

    # ---- gating from the x image tail: per q-half (batches {2q, 2q+1}),
    # logits[b, e] = sum_{d,t} x[b, d, L-6+t] * w_gate[d*5+t, e]
    gtrs = []
    for q in range(2):
        lg = psum_s.tile([2, E], f32, tag="s", name=f"lg{q}")
        half = xf[D * q : D * q + D, :].rearrange("d (p c) -> d c p", p=2)
        for t in range(5):
            nc.tensor.matmul(
                lg[:],
                half[:, L - 6 + t : L - 5 + t, :].bitcast(f32),
                cf[D * q : D * q + D, C_WG + E * t : C_WG + E * t + E],
                start=(t == 0),
                stop=(t == 4),
            )
        gtrs.append(_softmax_top2(nc, sm, lg, f32, AX, OP, AF, q))
    gT = sm.tile([E, NB], FAST_DT)  # col b = 2q + p
    for q in range(2):
        nc.vector.tensor_copy(gT[:, 2 * q : 2 * q + 2], gtrs[q][0:E, 0:2])

    # ---- bias first (it gates every PSUM drain in the main loop):
    # bT[oc, b] = sum_e c2b[e, oc] * gates[b, e], transposed on the PE.
    wp3 = psum_s.tile([OC, NB], f32, tag="s")
    nc.tensor.matmul(wp3[:], c2b[:], gT[:], start=True, stop=True)
    bT = sm.tile([OC, NB], f32)
    nc.vector.tensor_copy(bT[:], wp3[:])
    beffd = const.tile([2 * OC, 2], f32)
    for qq in range(2):
        nc.sync.dma_start(
            beffd[32 * qq : 32 * qq + 32, :], bT[0:OC, 2 * qq : 2 * qq + 2]
        )
    beffs = [beffd[:, p : p + 1] for p in range(2)]

    # ---- W_eff[b] = gates[b] @ c2: weff[b, ic*32+oc]
    weff = const.tile([NB, OC * OC], f32)
    wp1 = psum_s.tile([NB, 512], f32, tag="s")
    nc.tensor.matmul(wp1[:], gT[:], c2w[:, 0:512], start=True, stop=True)
    nc.vector.tensor_copy(weff[:, 0:512], wp1[:])
    wp2 = psum_s.tile([NB, 512], f32, tag="s")
    nc.tensor.matmul(wp2[:], gT[:], c2w[:, 512:1024], start=True, stop=True)
    nc.scalar.copy(weff[:, 512:1024], wp2[:])

    # Block-diagonal combine weights, one tile for both pairs:
    #   weTd[32q'+ic, 64p + 32q' + oc] = W_eff[2q'+p][oc, ic]
    # via a DRAM bounce (SBUF APs must not cross partitions mid-dim; the
    # DRAM-side strided reads are fine) with ONE scatter DMA per q'-half.
    # beffd[32q'+oc, p] = b_eff[2q'+p][oc] via 2 small SBUF->SBUF copies.
    wscr = dram.tile([NB, OC * OC], f32)
    nc.sync.dma_start(wscr[:], weff[:], max_dma_last_dim=OC * OC)
    weTd = const.tile([2 * OC, 2 * 2 * OC], FAST_DT)
    nc.vector.memset(weTd[:].bitcast(f32), 0.0)
    for qq in range(2):
        nc.sync.dma_start(
            weTd[32 * qq : 32 * qq + 32, :]
            .rearrange("ic (p oc) -> ic p oc", p=2)[:, :, 32 * qq : 32 * qq + 32],
            wscr[2 * qq : 2 * qq + 2, :]
            .rearrange("b (ic oc) -> ic b oc", ic=OC)
            .bitcast(FAST_DT),
        )
    weTs = [weTd[:, 64 * p : 64 * p + 64] for p in range(2)]

    # persistent output images: yb[p][32q + oc, c] = y[2q + p, oc, c]
    ybs = [
        const.tile([2 * OC, LP], f32, tag=f"yb{p}", name=f"yb{p}") for p in range(2)
    ]

    # ---- main loop over position tiles
    for t in range(NT):
        c0 = t * TS
        n = min(TS, LP - c0)
        for p in range(2):
            hp = psum_h.tile([2 * OC, TS], f32, tag="hp")
            for k in range(3):
                nc.tensor.matmul(
                    hp[:, 0:n],
                    w1t[:, 64 * k : 64 * k + 64],
                    xf[:, L * p + c0 + k : L * p + c0 + k + n],
                    start=(k == 0),
                    stop=(k == 2),
                )
            hs = hsb.tile([2 * OC, TS], FAST_DT, tag="hs")
            nc.scalar.activation(
                hs[:, 0:n], hp[:, 0:n], AF.Tanh, bias=b1p, scale=1.0
            )
            op_ = psum_o.tile([2 * OC, TS], f32, tag="op")
            nc.tensor.matmul(
                op_[:, 0:n], weTs[p], hs[:, 0:n], start=True, stop=True
            )
            # PSUM drain + b_eff add, alternating DVE / ACT to balance load
            if p == 0:
                nc.vector.tensor_scalar(
                    ybs[p][:, c0 : c0 + n], op_[:, 0:n], beffs[p],
                    None, op0=OP.add,
                )
            else:
                nc.scalar.add(ybs[p][:, c0 : c0 + n], op_[:, 0:n], beffs[p])

    # ---- stores: per batch, half-length (plain [32, n] SBUF slices)
    LH = LP // 2
    for hl in range(2):
        a0, a1 = (0, LH) if hl == 0 else (LH, LP)
        for p in range(2):
            for qq in range(2):
                nc.sync.dma_start(
                    y_d.ap()[2 * qq + p, :, a0:a1],
                    ybs[p][32 * qq : 32 * qq + 32, a0:a1],
                    max_dma_last_dim=a1 - a0,
                )


def _build():
    if "nc" in _CACHE:
        return _CACHE["nc"]
    nc = bacc.Bacc(
        "TRN2",
        target_bir_lowering=False,
        debug=False,
        num_devices=NCORES,
        detect_race_conditions=False,
    )
    f32 = mybir.dt.float32
    x_d = nc.dram_tensor("x", [NB, D, L], f32, kind="ExternalInput")
    cr_d = nc.dram_tensor("cr", [2 * D, NW1T], f32, kind="ExternalInput")
    cf_d = nc.dram_tensor("cf", [128, NCONST_F], f32, kind="ExternalInput")
    c2r_d = nc.dram_tensor("c2r", [E, NC2], f32, kind="ExternalInput")
    y_d = nc.dram_tensor("y", [NB, OC, LP], f32, kind="ExternalOutput")

    with tile.TileContext(nc) as tc:
        with ExitStack() as ctx:
            _emit(ctx, tc, nc, x_d, cr_d, cf_d, c2r_d, y_d)
    nc.compile()
    _CACHE["nc"] = nc
    return nc


def _prep_weights(w_gate, conv1_w, conv1_b, conv2_w, conv2_b):
    w_gate = np.asarray(w_gate, np.float32)
    conv1_w = np.asarray(conv1_w, np.float32)
    conv1_b = np.asarray(conv1_b, np.float32)
    conv2_w = np.asarray(conv2_w, np.float32)
    conv2_b = np.asarray(conv2_b, np.float32)
    # fp32r image: block-diagonal conv1 weights for 2-batch-stacked matmuls
    cr = np.zeros((2 * D, NW1T), np.float32)
    wkt = conv1_w.transpose(1, 2, 0)  # [d, k, oc]
    for k in range(3):
        cr[0:D, 64 * k : 64 * k + OC] = wkt[:, k, :]
        cr[D : 2 * D, 64 * k + OC : 64 * k + 2 * OC] = wkt[:, k, :]
    # f32 image
    cf = np.zeros((128, NCONST_F), np.float32)
    wgr = w_gate.reshape(D, 5 * E)
    cf[0:D, C_WG : C_WG + 5 * E] = wgr
    cf[D : 2 * D, C_WG : C_WG + 5 * E] = wgr  # duplicate for partition half q=1
    cf[0 : 2 * OC, C_B1P] = np.tile(conv1_b, 2)
    # fp32r conv2 image: c2w[e, ic*32+oc] = conv2_w[oc*8+e, ic, 0]
    c2 = np.zeros((E, NC2), np.float32)
    c2[:, C2_W : C2_W + OC * OC] = (
        conv2_w[:, :, 0].reshape(OC, E, OC).transpose(1, 2, 0).reshape(E, OC * OC)
    )
    c2[:, C2_B : C2_B + OC] = conv2_b.reshape(OC, E).T
    return np.ascontiguousarray(cr), np.ascontiguousarray(cf), np.ascontiguousarray(c2)


def _run(x, w_gate, conv1_w, conv1_b, conv2_w, conv2_b, **spmd_kwargs):
    x = np.asarray(x, np.float32)
    assert x.shape == (B, D, L), x.shape
    cr, cf, c2 = _prep_weights(w_gate, conv1_w, conv1_b, conv2_w, conv2_b)
    nc = _build()
    in_maps = []
    for i in range(NCORES):
        in_maps.append(
            {
                "x": np.ascontiguousarray(x[NB * i : NB * (i + 1)]),
                "cr": cr,
                "cf": cf,
                "c2r": c2,
            }
        )
    res = bass_utils.run_bass_kernel_spmd(
        nc, in_maps, core_ids=list(range(NCORES)), **spmd_kwargs
    )
    y = np.concatenate([r["y"] for r in res.results], axis=0)
    return np.ascontiguousarray(y.astype(np.float32)), res


def kernel(x, w_gate, conv1_w, conv1_b, conv2_w, conv2_b):
    y, _ = _run(x, w_gate, conv1_w, conv1_b, conv2_w, conv2_b)
    return y



# revision 13
# speedup vs baseline: 1.0375x; 1.0375x over previous
"""MoE-routed conv kernel (Channel_Embedding ablation) for 8 trn2 NeuronCores.

Math (see reference):
  gates  = top2-renormalized softmax( x[:, :, -6:-1].reshape(B, D*5) @ w_gate )
  h      = tanh(conv1d(x, conv1_w, VALID) + conv1_b)            # [B, OC, L-2]
  out    = conv1d(h, conv2_w, 1x1) + conv2_b                    # [B, OC*E, L-2]
  y[b,oc,t] = sum_e gates[b,e] * out[b, oc*E+e, t]

Key algebraic fold: the expert combine commutes with the 1x1 conv, so per
batch element
  W_eff[b][oc, ic] = sum_e gates[b,e] * conv2_w[oc*E+e, ic, 0]
  b_eff[b][oc]     = sum_e gates[b,e] * conv2_b[oc*E+e]
  y[b] = W_eff[b] @ h[b] + b_eff[b]

Sharding: data-parallel over batch B=32 across 8 cores (4 each); weights
replicated.

Layout (per core, batches b = 2q + p, pair p, half q):
  - x is shipped bf16 as xb[p, 64q+d, c]; the rel-err budget (2e-2) is ~5x
    above the bf16 conv error (~4e-3). The 5-col gating window ships
    separately in fp32 so top-2 expert selection is exact.
  - conv1 weights are six zero-padded [128,128] bf16 blocks (tap k, pair p):
    rows 64q+d -> out cols 64p+32q+oc. Since matmul cost is per moving
    column (M is free), the zero half makes pair p's outputs land on PSUM
    partitions 64p..64p+31+32, so BOTH pairs accumulate into ONE PSUM bank
    (engines are lane-locked; this is what enables the merged layout).
  - per position tile [512]: 6 conv matmuls -> one PSUM [128,512] ->
    one tanh (+conv1 bias) -> hs bf16 [128,512] -> ONE combine matmul with
    block-diag Wc [128,128] bf16 (K=128, all 4 batches at once) -> one DVE
    drain (+b_eff) into a persistent y image [128, 4094] -> 3 large column-
    range stores (one per (p,q) slice), issued as halves complete.
  - Wc is built from W_eff = gates @ c2w via a small DRAM bounce with 4
    diagonal-block scatter reads; b_eff via one PE matmul + 4 tiny copies.

DMA budget: loads (4 x 512KB, 4KB rows) on Sync HWDGE; constants + Wc
scatter on Scalar HWDGE (second ring); bounce write on GpSimd SWDGE; the
3 y stores (8-16KB rows) on Sync. Each dma_start costs ~0.65us of issue
time on its sequencer, so counts are kept minimal and split across rings.
"""

from contextlib import ExitStack

import numpy as np
import ml_dtypes

import concourse.bacc as bacc
import concourse.mybir as mybir
import concourse.tile as tile
from concourse import bass_utils

B, D, L = 32, 64, 4096
E, TOPK, OC = 8, 2, 32
LP = L - 2  # 4094 valid conv outputs
NCORES = 8
NB = B // NCORES  # batch elements per core
TS = 512  # position tile (one PSUM bank of fp32)
NT = (LP + TS - 1) // TS

F32R = mybir.dt.float32r  # fp32 bits, fast PE mode (for tiny W_eff matmuls)
BF16 = mybir.dt.bfloat16

# f32 constants image [128, NCONST_F]:
#   wg   [*, 40] gating weights, duplicated in both partition halves,
#        col = 8*t + e
#   gwin [*, 10] gating window x[2q+p, d, 4090+t] at col = 2*t + p
#   b1t  [*, 1]  conv1 bias by output row (64p+32q+oc -> conv1_b[oc])
C_WG = 0
C_GWIN = C_WG + 5 * E
C_B1 = C_GWIN + 10
NCONST_F = C_B1 + 1
# f32 conv2 image [8, NC2]: c2w[e, 32*ic+oc]
C2_W, NC2 = 0, OC * OC
# bf16 weight image [128, NWB]: conv1 blocks j = 3*p + k at cols 128j,
# then c2b_rep[e, 32*bb+oc] (rows 0:8) at cols NW1 (bf16 so the b_eff
# matmul can have M=128; fp32r pairs PE column groups and caps M at 64)
NW1 = 6 * 128
C2_B = NW1
NWB = NW1 + 4 * OC

_CACHE: dict = {}


def _softmax_top2(nc, sm, lg, f32, AX, OP, AF, q):
    """Per-half gating: lg [2, E] logits (PSUM) -> gates.T [32, 32] in SBUF.

    gates = (e >= m2) * e / (m1 + m2), e = exp(logits) - identical to
    softmax -> top2 -> vk/(sum vk + 1e-6) up to the 1e-6 term (negligible).
    """
    e_sb = sm.tile([2, E], f32, name=f"e_sb{q}")
    nc.scalar.activation(e_sb[:], lg[:], AF.Exp)
    m1 = sm.tile([2, 1], f32, name=f"m1_{q}")
    nc.vector.reduce_max(m1[:], e_sb[:], axis=AX.X)
    lt = sm.tile([2, E], f32, name=f"lt{q}")
    nc.vector.tensor_scalar(lt[:], e_sb[:], m1[:], None, op0=OP.is_lt)
    emsk = sm.tile([2, E], f32, name=f"emsk{q}")
    nc.vector.tensor_mul(emsk[:], lt[:], e_sb[:])  # e with the max zeroed
    m2 = sm.tile([2, 1], f32, name=f"m2_{q}")
    nc.vector.reduce_max(m2[:], emsk[:], axis=AX.X)
    den3 = sm.tile([2, 1], f32, name=f"den3{q}")
    nc.vector.tensor_add(den3[:], m1[:], m2[:])
    rcp = sm.tile([2, 1], f32, name=f"rcp{q}")
    nc.vector.reciprocal(rcp[:], den3[:])
    ge = sm.tile([2, E], f32, name=f"ge{q}")
    nc.vector.tensor_scalar(ge[:], e_sb[:], m2[:], None, op0=OP.is_ge)
    gnum = sm.tile([2, E], f32, name=f"gnum{q}")
    nc.vector.tensor_mul(gnum[:], ge[:], e_sb[:])
    gpad = sm.tile([32, 32], f32, name=f"gpad{q}")
    nc.vector.memset(gpad[:], 0.0)
    nc.vector.tensor_scalar(gpad[0:2, 0:E], gnum[:], rcp[:], None, op0=OP.mult)
    gtr = sm.tile([32, 32], f32, name=f"gtr{q}")
    nc.vector.transpose(gtr[:], gpad[:])  # 32x32 block transpose
    return gtr  # gtr[0:E, 0:2] = gates.T for batches {2q, 2q+1}


def _emit(ctx, tc, nc, x_d, cw_d, cf_d, c2_d, y_d):
    f32 = mybir.dt.float32
    AF = mybir.ActivationFunctionType
    AX = mybir.AxisListType
    OP = mybir.AluOpType

    const = ctx.enter_context(tc.tile_pool(name="const", bufs=1))
    sm = ctx.enter_context(tc.tile_pool(name="sm", bufs=1))
    hsb = ctx.enter_context(tc.tile_pool(name="hsb", bufs=3))
    psum_h = ctx.enter_context(tc.tile_pool(name="ph", bufs=3, space="PSUM"))
    psum_y = ctx.enter_context(tc.tile_pool(name="py", bufs=2, space="PSUM"))
    psum_s = ctx.enter_context(tc.tile_pool(name="ps", bufs=2, space="PSUM"))
    dram = ctx.enter_context(tc.tile_pool(name="dram", bufs=1, space="DRAM"))

    # ---- x image loads on Sync HWDGE: xf[64q+d, 4096p+c] = x[2q+p, d, c],
    # 4KB-per-row descriptors, in consumption order (p0 first, p1 second).
    xf = const.tile([2 * D, 2 * L], BF16)
    for ch, (p, a0, a1) in enumerate(
        [(0, 0, 2048), (1, 0, 2048), (0, 2048, L), (1, 2048, L)]
    ):
        nc.sync.dma_start(
            xf[:, L * p + a0 : L * p + a1],
            x_d.ap()[p, :, a0:a1],
        )

    # ---- constants on Scalar HWDGE (second ring; does not delay x loads)
    cf = const.tile([128, NCONST_F], f32)
    nc.scalar.dma_start(cf[:], cf_d.ap())
    c2 = const.tile([E, NC2], F32R)
    nc.scalar.dma_start(c2[:], c2_d.ap().bitcast(F32R))
    cw = const.tile([128, NWB], BF16)
    nc.scalar.dma_start(cw[:], cw_d.ap())
    b1t = cf[:, C_B1 : C_B1 + 1]

    # ---- ACT table warmup (exp/tanh share one table set; load it early)
    warm = sm.tile([1, 8], f32)
    nc.vector.memset(warm[:], 0.0)
    warm2 = sm.tile([1, 8], f32)
    nc.scalar.activation(warm2[:], warm[:], AF.Exp)

    # ---- PE warmup: dummy matmuls (no data deps) so the PE clock ramp
    # happens during the load window, not on the first real matmuls.
    wsrc = sm.tile([128, 128], BF16)
    nc.vector.memset(wsrc[:].bitcast(f32), 0.0)
    wup = psum_s.tile([128, TS], f32, tag="s")
    for _ in range(10):
        nc.tensor.matmul(wup[:, 0:128], wsrc[:], wsrc[:], start=True, stop=True)

    # ---- gating from the fp32 window: per q-half (batches {2q, 2q+1}),
    # logits[p, e] = sum_{d,t} gwin[64q+d, 2t+p] * wg[64q+d, 8t+e]
    gtrs = []
    for q in range(2):
        lg = psum_s.tile([2, E], f32, tag="s", name=f"lg{q}")
        for t in range(5):
            nc.tensor.matmul(
                lg[:],
                cf[D * q : D * q + D, C_GWIN + 2 * t : C_GWIN + 2 * t + 2],
                cf[D * q : D * q + D, C_WG + E * t : C_WG + E * t + E],
                start=(t == 0),
                stop=(t == 4),
            )
        gtrs.append(_softmax_top2(nc, sm, lg, f32, AX, OP, AF, q))
    gT = sm.tile([E, NB], F32R)  # col b = 2q + p
    gTb = sm.tile([E, NB], BF16)
    for q in range(2):
        nc.vector.tensor_copy(gT[:, 2 * q : 2 * q + 2], gtrs[q][0:E, 0:2])
        nc.vector.tensor_copy(gTb[:, 2 * q : 2 * q + 2], gtrs[q][0:E, 0:2])
    # column view in bb = 2p + q order (matches output row blocks 32bb+oc)
    gTbb = gTb[:].rearrange("e (q p) -> e p q", q=2)  # 3D: col order bb = 2p+q

    # ---- b_eff[128,1]: rows 32bb+oc. One PE matmul gives
    # bp[32bb+oc, bb'] = sum_e c2b_rep[e, 32bb+oc] * gates[b(bb'), e];
    # the diagonal blocks are extracted with 4 tiny copies.
    bp = psum_s.tile([OC * NB, NB], f32, tag="s")
    nc.tensor.matmul(
        bp[:],
        cw[0:E, C2_B : C2_B + 4 * OC],
        gTbb,
        start=True,
        stop=True,
    )
    beff = const.tile([OC * NB, 1], f32)
    for bb in range(NB):
        nc.vector.tensor_copy(
            beff[OC * bb : OC * bb + OC, :], bp[OC * bb : OC * bb + OC, bb : bb + 1]
        )

    # ---- W_eff[b] = gates[b] @ c2w -> weffb[b, 32*ic+oc] (bf16), then a
    # DRAM bounce scatters the four [32,32] diagonal blocks into Wc.
    weffb = sm.tile([NB, OC * OC], BF16)
    for hh in range(2):
        wp = psum_s.tile([NB, 512], f32, tag="s", name=f"wp{hh}")
        nc.tensor.matmul(
            wp[:],
            gT[:],
            c2[:, C2_W + 512 * hh : C2_W + 512 * (hh + 1)],
            start=True,
            stop=True,
        )
        nc.vector.tensor_copy(weffb[:, 512 * hh : 512 * (hh + 1)], wp[:])
    wscr = dram.tile([NB, OC * OC], BF16)
    nc.gpsimd.dma_start(wscr[:], weffb[:])
    Wc = const.tile([128, 128], BF16)
    nc.vector.memset(Wc[:].bitcast(f32), 0.0)
    for bb in range(NB):
        p, q = bb // 2, bb % 2
        b = 2 * q + p
        nc.scalar.dma_start(
            Wc[OC * bb : OC * bb + OC, OC * bb : OC * bb + OC],
            wscr[b : b + 1, :].rearrange("one (ic oc) -> (one ic) oc", ic=OC),
        )

    # persistent output image: yb[64p+32q+oc, c] = y[2q+p, oc, c]
    yb = const.tile([128, LP], f32)

    # ---- main loop over position tiles, software-pipelined one stage so
    # the first combine (needs Wc ~5us in) trails the first conv.
    stage = []  # (hp, hs, c0, n) pending tanh/combine/drain

    def finish(hp, hs, c0, n):
        nc.scalar.activation(hs[:, 0:n], hp[:, 0:n], AF.Tanh, bias=b1t, scale=1.0)
        op_ = psum_y.tile([128, TS], f32, tag="op")
        nc.tensor.matmul(op_[:, 0:n], Wc[:], hs[:, 0:n], start=True, stop=True)
        nc.vector.tensor_scalar(
            yb[:, c0 : c0 + n], op_[:, 0:n], beff[:], None, op0=OP.add
        )

    def store(a0, a1):
        # one DMA per (p, q) slice: DRAM rows are (a1-a0)*4B contiguous
        for p in range(2):
            for q in range(2):
                nc.sync.dma_start(
                    y_d.ap()[2 * q + p, :, a0:a1],
                    yb[64 * p + 32 * q : 64 * p + 32 * q + 32, a0:a1],
                )

    for t in range(NT):
        c0 = t * TS
        n = min(TS, LP - c0)
        hp = psum_h.tile([128, TS], f32, tag="hp")
        for p in range(2):
            for k in range(3):
                nc.tensor.matmul(
                    hp[:, 0:n],
                    cw[:, 128 * (3 * p + k) : 128 * (3 * p + k) + 128],
                    xf[:, L * p + c0 + k : L * p + c0 + k + n],
                    start=(p == 0 and k == 0),
                    stop=(p == 1 and k == 2),
                )
        hs = hsb.tile([128, TS], BF16, tag="hs")
        stage.append((hp, hs, c0, n))
        if len(stage) > 1:
            finish(*stage.pop(0))
        if t == 4:
            store(0, 4 * TS)  # tiles 0-3 drained by now (pipeline lag 1)
        elif t == 7:
            store(4 * TS, 7 * TS)
    finish(*stage.pop(0))
    store(7 * TS, LP)


def _build():
    if "nc" in _CACHE:
        return _CACHE["nc"]
    nc = bacc.Bacc(
        "TRN2",
        target_bir_lowering=False,
        debug=False,
        num_devices=NCORES,
        detect_race_conditions=False,
    )
    f32 = mybir.dt.float32
    x_d = nc.dram_tensor("x", [2, 2 * D, L], BF16, kind="ExternalInput")
    cw_d = nc.dram_tensor("cw", [128, NWB], BF16, kind="ExternalInput")
    cf_d = nc.dram_tensor("cf", [128, NCONST_F], f32, kind="ExternalInput")
    c2_d = nc.dram_tensor("c2", [E, NC2], f32, kind="ExternalInput")
    y_d = nc.dram_tensor("y", [NB, OC, LP], f32, kind="ExternalOutput")

    with tile.TileContext(nc) as tc:
        with ExitStack() as ctx:
            _emit(ctx, tc, nc, x_d, cw_d, cf_d, c2_d, y_d)
    nc.compile()
    _CACHE["nc"] = nc
    return nc


def _prep_weights(w_gate, conv1_w, conv1_b, conv2_w, conv2_b):
    w_gate = np.asarray(w_gate, np.float32)
    conv1_w = np.asarray(conv1_w, np.float32)
    conv1_b = np.asarray(conv1_b, np.float32)
    conv2_w = np.asarray(conv2_w, np.float32)
    conv2_b = np.asarray(conv2_b, np.float32)
    # bf16 conv1 blocks: cw[:, 128j:128j+128], j = 3p+k:
    #   rows 64q+d -> cols 64p+32q+oc = conv1_w[oc, d, k]
    cw = np.zeros((128, NWB), np.float32)
    wkt = conv1_w.transpose(1, 2, 0)  # [d, k, oc]
    for p in range(2):
        for k in range(3):
            j = 3 * p + k
            for q in range(2):
                cw[
                    D * q : D * q + D,
                    128 * j + 64 * p + 32 * q : 128 * j + 64 * p + 32 * q + OC,
                ] = wkt[:, k, :]
    # c2b_rep[e, 32*bb+oc] = conv2_b[oc*8+e] for every bb
    cw[0:E, C2_B : C2_B + 4 * OC] = np.tile(conv2_b.reshape(OC, E).T, (1, 4))
    cw = cw.astype(ml_dtypes.bfloat16)
    # f32 image (gwin filled per-core later)
    cf = np.zeros((128, NCONST_F), np.float32)
    wgr = w_gate.reshape(D, 5 * E)  # [d, t*E + e]
    cf[0:D, C_WG : C_WG + 5 * E] = wgr
    cf[D : 2 * D, C_WG : C_WG + 5 * E] = wgr
    cf[:, C_B1] = np.tile(conv1_b, 4)  # rows 64p+32q+oc -> conv1_b[oc]
    # f32 conv2 image: c2w[e, 32*ic+oc] = conv2_w[oc*8+e, ic, 0]
    c2 = np.zeros((E, NC2), np.float32)
    c2[:, C2_W : C2_W + OC * OC] = (
        conv2_w[:, :, 0].reshape(OC, E, OC).transpose(1, 2, 0).reshape(E, OC * OC)
    )
    return np.ascontiguousarray(cw), cf, np.ascontiguousarray(c2)


def _run(x, w_gate, conv1_w, conv1_b, conv2_w, conv2_b, **spmd_kwargs):
    x = np.asarray(x, np.float32)
    assert x.shape == (B, D, L), x.shape
    cw, cf, c2 = _prep_weights(w_gate, conv1_w, conv1_b, conv2_w, conv2_b)
    nc = _build()
    # xb[core, p, 64q+d, c] = x[4*core + 2q+p, d, c], bf16
    xr = x.reshape(NCORES, 2, 2, D, L)  # [core, q, p, d, c]
    xb = np.ascontiguousarray(xr.transpose(0, 2, 1, 3, 4)).reshape(
        NCORES, 2, 2 * D, L
    )
    xb16 = xb.astype(ml_dtypes.bfloat16)
    in_maps = []
    for i in range(NCORES):
        cfi = cf.copy()
        # gwin: cf[64q+d, C_GWIN + 2t + p] = x[2q+p, d, 4090+t] (fp32)
        win = xb[i, :, :, L - 6 : L - 1]  # [p, 64q+d, t]
        cfi[:, C_GWIN : C_GWIN + 10] = win.transpose(1, 2, 0).reshape(2 * D, 10)
        in_maps.append(
            {
                "x": np.ascontiguousarray(xb16[i]),
                "cw": cw,
                "cf": cfi,
                "c2": c2,
            }
        )
    res = bass_utils.run_bass_kernel_spmd(
        nc, in_maps, core_ids=list(range(NCORES)), **spmd_kwargs
    )
    y = np.concatenate([r["y"] for r in res.results], axis=0)
    return np.ascontiguousarray(y.astype(np.float32)), res


def kernel(x, w_gate, conv1_w, conv1_b, conv2_w, conv2_b):
    y, _ = _run(x, w_gate, conv1_w, conv1_b, conv2_w, conv2_b)
    return y


# revision 15
# speedup vs baseline: 1.0604x; 1.0221x over previous
"""MoE-routed conv kernel (Channel_Embedding ablation) for 8 trn2 NeuronCores.

Math (see reference):
  gates  = top2-renormalized softmax( x[:, :, -6:-1].reshape(B, D*5) @ w_gate )
  h      = tanh(conv1d(x, conv1_w, VALID) + conv1_b)            # [B, OC, L-2]
  out    = conv1d(h, conv2_w, 1x1) + conv2_b                    # [B, OC*E, L-2]
  y[b,oc,t] = sum_e gates[b,e] * out[b, oc*E+e, t]

Key algebraic fold: the expert combine commutes with the 1x1 conv, so per
batch element
  W_eff[b][oc, ic] = sum_e gates[b,e] * conv2_w[oc*E+e, ic, 0]
  b_eff[b][oc]     = sum_e gates[b,e] * conv2_b[oc*E+e]
  y[b] = W_eff[b] @ h[b] + b_eff[b]

Sharding: data-parallel over batch B=32 across 8 cores (4 each); weights
replicated.

Layout (per core, batches b = 2q + p, pair p, half q):
  - x is shipped bf16 as xb[p, 64q+d, c]; the rel-err budget (2e-2) is ~5x
    above the bf16 conv error (~4e-3). The 5-col gating window ships
    separately in fp32 so top-2 expert selection is exact.
  - conv1 weights are six zero-padded [128,128] bf16 blocks (tap k, pair p):
    rows 64q+d -> out cols 64p+32q+oc. Since matmul cost is per moving
    column (M is free), the zero half makes pair p's outputs land on PSUM
    partitions 64p..64p+31+32, so BOTH pairs accumulate into ONE PSUM bank
    (engines are lane-locked; this is what enables the merged layout).
  - per position tile [512]: 6 conv matmuls -> one PSUM [128,512] ->
    one tanh (+conv1 bias) -> hs bf16 [128,512] -> ONE combine matmul with
    block-diag Wc [128,128] bf16 (K=128, all 4 batches at once) -> one DVE
    drain (+b_eff) into a persistent y image [128, 4094] -> 3 large column-
    range stores (one per (p,q) slice), issued as halves complete.
  - Wc is built from W_eff = gates @ c2w via a small DRAM bounce with 4
    diagonal-block scatter reads; b_eff via one PE matmul + 4 tiny copies.

DMA budget: loads (4 x 512KB, 4KB rows) on Sync HWDGE; constants + Wc
scatter on Scalar HWDGE (second ring); bounce write on GpSimd SWDGE; the
3 y stores (8-16KB rows) on Sync. Each dma_start costs ~0.65us of issue
time on its sequencer, so counts are kept minimal and split across rings.
"""

from contextlib import ExitStack

import numpy as np
import ml_dtypes

import concourse.bacc as bacc
import concourse.mybir as mybir
import concourse.tile as tile
from concourse import bass_utils

B, D, L = 32, 64, 4096
E, TOPK, OC = 8, 2, 32
LP = L - 2  # 4094 valid conv outputs
NCORES = 8
NB = B // NCORES  # batch elements per core
TS = 512  # position tile (one PSUM bank of fp32)
NT = (LP + TS - 1) // TS

F32R = mybir.dt.float32r  # fp32 bits, fast PE mode (for tiny W_eff matmuls)
BF16 = mybir.dt.bfloat16

# f32 constants image [128, NCONST_F]:
#   wg   [*, 40] gating weights, duplicated in both partition halves,
#        col = 8*t + e
#   gwin [*, 10] gating window x[2q+p, d, 4090+t] at col = 2*t + p
#   b1t  [*, 1]  conv1 bias by output row (64p+32q+oc -> conv1_b[oc])
C_WG = 0
C_GWIN = C_WG + 5 * E
C_B1 = C_GWIN + 10
NCONST_F = C_B1 + 1
# f32 conv2 image [8, NC2]: c2w[e, 32*ic+oc]
C2_W, NC2 = 0, OC * OC
# bf16 weight image [128, NWB]: conv1 blocks j = 3*p + k at cols 128j,
# then c2b_rep[e, 32*bb+oc] (rows 0:8) at cols NW1 (bf16 so the b_eff
# matmul can have M=128; fp32r pairs PE column groups and caps M at 64)
NW1 = 6 * 128
C2_B = NW1
NWB = NW1 + 4 * OC

_CACHE: dict = {}


def _softmax_top2(nc, sm, lg, f32, AX, OP, AF, q):
    """Per-half gating: lg [2, E] logits (PSUM) -> gates.T [32, 32] in SBUF.

    gates = (e >= m2) * e / (m1 + m2), e = exp(logits) - identical to
    softmax -> top2 -> vk/(sum vk + 1e-6) up to the 1e-6 term (negligible).
    """
    e_sb = sm.tile([2, E], f32, name=f"e_sb{q}")
    nc.scalar.activation(e_sb[:], lg[:], AF.Exp)
    m1 = sm.tile([2, 1], f32, name=f"m1_{q}")
    nc.vector.reduce_max(m1[:], e_sb[:], axis=AX.X)
    lt = sm.tile([2, E], f32, name=f"lt{q}")
    nc.vector.tensor_scalar(lt[:], e_sb[:], m1[:], None, op0=OP.is_lt)
    emsk = sm.tile([2, E], f32, name=f"emsk{q}")
    nc.vector.tensor_mul(emsk[:], lt[:], e_sb[:])  # e with the max zeroed
    m2 = sm.tile([2, 1], f32, name=f"m2_{q}")
    nc.vector.reduce_max(m2[:], emsk[:], axis=AX.X)
    den3 = sm.tile([2, 1], f32, name=f"den3{q}")
    nc.vector.tensor_add(den3[:], m1[:], m2[:])
    rcp = sm.tile([2, 1], f32, name=f"rcp{q}")
    nc.vector.reciprocal(rcp[:], den3[:])
    ge = sm.tile([2, E], f32, name=f"ge{q}")
    nc.vector.tensor_scalar(ge[:], e_sb[:], m2[:], None, op0=OP.is_ge)
    gnum = sm.tile([2, E], f32, name=f"gnum{q}")
    nc.vector.tensor_mul(gnum[:], ge[:], e_sb[:])
    gpad = sm.tile([32, 32], f32, name=f"gpad{q}")
    nc.vector.memset(gpad[:], 0.0)
    nc.vector.tensor_scalar(gpad[0:2, 0:E], gnum[:], rcp[:], None, op0=OP.mult)
    gtr = sm.tile([32, 32], f32, name=f"gtr{q}")
    nc.vector.transpose(gtr[:], gpad[:])  # 32x32 block transpose
    return gtr  # gtr[0:E, 0:2] = gates.T for batches {2q, 2q+1}


def _emit(ctx, tc, nc, x_d, cw_d, cf_d, c2_d, y_d):
    f32 = mybir.dt.float32
    AF = mybir.ActivationFunctionType
    AX = mybir.AxisListType
    OP = mybir.AluOpType

    const = ctx.enter_context(tc.tile_pool(name="const", bufs=1))
    sm = ctx.enter_context(tc.tile_pool(name="sm", bufs=1))
    # all 8 hs tiles stay live so tanh (and thus conv PSUM recycling) never
    # stalls on combines, which wait for Wc (~13us) early in the run
    hsb = ctx.enter_context(tc.tile_pool(name="hsb", bufs=8))
    psum_h = ctx.enter_context(tc.tile_pool(name="ph", bufs=3, space="PSUM"))
    psum_y = ctx.enter_context(tc.tile_pool(name="py", bufs=3, space="PSUM"))
    psum_s = ctx.enter_context(tc.tile_pool(name="ps", bufs=2, space="PSUM"))
    dram = ctx.enter_context(tc.tile_pool(name="dram", bufs=1, space="DRAM"))

    # ---- x image loads on Sync HWDGE: xf[64q+d, 4096p+c] = x[2q+p, d, c],
    # 4KB-per-row descriptors, in consumption order (p0 first, p1 second).
    xf = const.tile([2 * D, 2 * L], BF16)
    for ch, (p, a0, a1) in enumerate(
        [(0, 0, 2048), (1, 0, 2048), (0, 2048, L), (1, 2048, L)]
    ):
        nc.sync.dma_start(
            xf[:, L * p + a0 : L * p + a1],
            x_d.ap()[p, :, a0:a1],
        )

    # ---- constants on Scalar HWDGE (second ring; does not delay x loads)
    cf = const.tile([128, NCONST_F], f32)
    nc.scalar.dma_start(cf[:], cf_d.ap())
    c2 = const.tile([E, NC2], F32R)
    nc.scalar.dma_start(c2[:], c2_d.ap().bitcast(F32R))
    cw = const.tile([128, NWB], BF16)
    nc.scalar.dma_start(cw[:], cw_d.ap())
    b1t = cf[:, C_B1 : C_B1 + 1]

    # ---- ACT table warmup (exp/tanh share one table set; load it early)
    warm = sm.tile([1, 8], f32)
    nc.vector.memset(warm[:], 0.0)
    warm2 = sm.tile([1, 8], f32)
    nc.scalar.activation(warm2[:], warm[:], AF.Exp)

    # ---- PE warmup: dummy matmuls (no data deps) so the PE clock ramp
    # happens during the load window, not on the first real matmuls.
    wsrc = sm.tile([128, 128], BF16)
    nc.vector.memset(wsrc[:].bitcast(f32), 0.0)
    wup = psum_s.tile([128, TS], f32, tag="s")
    for _ in range(10):
        nc.tensor.matmul(wup[:, 0:128], wsrc[:], wsrc[:], start=True, stop=True)

    # ---- gating from the fp32 window: per q-half (batches {2q, 2q+1}),
    # logits[p, e] = sum_{d,t} gwin[64q+d, 2t+p] * wg[64q+d, 8t+e]
    gtrs = []
    for q in range(2):
        lg = psum_s.tile([2, E], f32, tag="s", name=f"lg{q}")
        for t in range(5):
            nc.tensor.matmul(
                lg[:],
                cf[D * q : D * q + D, C_GWIN + 2 * t : C_GWIN + 2 * t + 2],
                cf[D * q : D * q + D, C_WG + E * t : C_WG + E * t + E],
                start=(t == 0),
                stop=(t == 4),
            )
        gtrs.append(_softmax_top2(nc, sm, lg, f32, AX, OP, AF, q))
    gT = sm.tile([E, NB], F32R)  # col b = 2q + p
    gTb = sm.tile([E, NB], BF16)
    for q in range(2):
        nc.vector.tensor_copy(gT[:, 2 * q : 2 * q + 2], gtrs[q][0:E, 0:2])
        nc.vector.tensor_copy(gTb[:, 2 * q : 2 * q + 2], gtrs[q][0:E, 0:2])
    # column view in bb = 2p + q order (matches output row blocks 32bb+oc)
    gTbb = gTb[:].rearrange("e (q p) -> e p q", q=2)  # 3D: col order bb = 2p+q

    # ---- b_eff[128,1]: rows 32bb+oc. One PE matmul gives
    # bp[32bb+oc, bb'] = sum_e c2b_rep[e, 32bb+oc] * gates[b(bb'), e];
    # the diagonal blocks are extracted with 4 tiny copies.
    bp = psum_s.tile([OC * NB, NB], f32, tag="s")
    nc.tensor.matmul(
        bp[:],
        cw[0:E, C2_B : C2_B + 4 * OC],
        gTbb,
        start=True,
        stop=True,
    )
    beff = const.tile([OC * NB, 1], f32)
    for bb in range(NB):
        nc.vector.tensor_copy(
            beff[OC * bb : OC * bb + OC, :], bp[OC * bb : OC * bb + OC, bb : bb + 1]
        )

    # ---- W_eff[b] = gates[b] @ c2w -> weffb[b, 32*ic+oc] (bf16), then a
    # DRAM bounce scatters the four [32,32] diagonal blocks into Wc.
    weffb = sm.tile([NB, OC * OC], BF16)
    for hh in range(2):
        wp = psum_s.tile([NB, 512], f32, tag="s", name=f"wp{hh}")
        nc.tensor.matmul(
            wp[:],
            gT[:],
            c2[:, C2_W + 512 * hh : C2_W + 512 * (hh + 1)],
            start=True,
            stop=True,
        )
        nc.vector.tensor_copy(weffb[:, 512 * hh : 512 * (hh + 1)], wp[:])
    # bounce DMAs all on Sync (issue after the x loads); Scalar stays free
    # for tanhs so they pipeline right behind the conv matmuls
    wscr = dram.tile([NB, OC * OC], BF16)
    nc.sync.dma_start(wscr[:], weffb[:])
    Wc = const.tile([128, 128], BF16)
    nc.vector.memset(Wc[:].bitcast(f32), 0.0)
    for bb in range(NB):
        p, q = bb // 2, bb % 2
        b = 2 * q + p
        nc.sync.dma_start(
            Wc[OC * bb : OC * bb + OC, OC * bb : OC * bb + OC],
            wscr[b : b + 1, :].rearrange("one (ic oc) -> (one ic) oc", ic=OC),
        )

    # persistent output image: yb[64p+32q+oc, c] = y[2q+p, oc, c]
    yb = const.tile([128, LP], f32)

    # ---- main loop over position tiles, software-pipelined one stage so
    # the first combine (needs Wc ~5us in) trails the first conv.
    stage = []  # (hp, hs, c0, n) pending tanh/combine/drain

    def finish(hp, hs, c0, n):
        nc.scalar.activation(hs[:, 0:n], hp[:, 0:n], AF.Tanh, bias=b1t, scale=1.0)
        op_ = psum_y.tile([128, TS], f32, tag="op")
        nc.tensor.matmul(op_[:, 0:n], Wc[:], hs[:, 0:n], start=True, stop=True)
        nc.vector.tensor_scalar(
            yb[:, c0 : c0 + n], op_[:, 0:n], beff[:], None, op0=OP.add
        )

    def store(a0, a1):
        # one DMA per (p, q) slice: DRAM rows are (a1-a0)*4B contiguous
        for p in range(2):
            for q in range(2):
                nc.sync.dma_start(
                    y_d.ap()[2 * q + p, :, a0:a1],
                    yb[64 * p + 32 * q : 64 * p + 32 * q + 32, a0:a1],
                )

    for t in range(NT):
        c0 = t * TS
        n = min(TS, LP - c0)
        hp = psum_h.tile([128, TS], f32, tag="hp")
        for p in range(2):
            for k in range(3):
                nc.tensor.matmul(
                    hp[:, 0:n],
                    cw[:, 128 * (3 * p + k) : 128 * (3 * p + k) + 128],
                    xf[:, L * p + c0 + k : L * p + c0 + k + n],
                    start=(p == 0 and k == 0),
                    stop=(p == 1 and k == 2),
                )
        hs = hsb.tile([128, TS], BF16, tag="hs")
        stage.append((hp, hs, c0, n))
        if len(stage) > 1:
            finish(*stage.pop(0))
        if t == 4:
            store(0, 4 * TS)  # tiles 0-3 drained by now (pipeline lag 1)
        elif t == 7:
            store(4 * TS, 7 * TS)
    finish(*stage.pop(0))
    store(7 * TS, LP)


def _build():
    if "nc" in _CACHE:
        return _CACHE["nc"]
    nc = bacc.Bacc(
        "TRN2",
        target_bir_lowering=False,
        debug=False,
        num_devices=NCORES,
        detect_race_conditions=False,
    )
    f32 = mybir.dt.float32
    x_d = nc.dram_tensor("x", [2, 2 * D, L], BF16, kind="ExternalInput")
    cw_d = nc.dram_tensor("cw", [128, NWB], BF16, kind="ExternalInput")
    cf_d = nc.dram_tensor("cf", [128, NCONST_F], f32, kind="ExternalInput")
    c2_d = nc.dram_tensor("c2", [E, NC2], f32, kind="ExternalInput")
    y_d = nc.dram_tensor("y", [NB, OC, LP], f32, kind="ExternalOutput")

    with tile.TileContext(nc) as tc:
        with ExitStack() as ctx:
            _emit(ctx, tc, nc, x_d, cw_d, cf_d, c2_d, y_d)
    nc.compile()
    _CACHE["nc"] = nc
    return nc


def _prep_weights(w_gate, conv1_w, conv1_b, conv2_w, conv2_b):
    w_gate = np.asarray(w_gate, np.float32)
    conv1_w = np.asarray(conv1_w, np.float32)
    conv1_b = np.asarray(conv1_b, np.float32)
    conv2_w = np.asarray(conv2_w, np.float32)
    conv2_b = np.asarray(conv2_b, np.float32)
    # bf16 conv1 blocks: cw[:, 128j:128j+128], j = 3p+k:
    #   rows 64q+d -> cols 64p+32q+oc = conv1_w[oc, d, k]
    cw = np.zeros((128, NWB), np.float32)
    wkt = conv1_w.transpose(1, 2, 0)  # [d, k, oc]
    for p in range(2):
        for k in range(3):
            j = 3 * p + k
            for q in range(2):
                cw[
                    D * q : D * q + D,
                    128 * j + 64 * p + 32 * q : 128 * j + 64 * p + 32 * q + OC,
                ] = wkt[:, k, :]
    # c2b_rep[e, 32*bb+oc] = conv2_b[oc*8+e] for every bb
    cw[0:E, C2_B : C2_B + 4 * OC] = np.tile(conv2_b.reshape(OC, E).T, (1, 4))
    cw = cw.astype(ml_dtypes.bfloat16)
    # f32 image (gwin filled per-core later)
    cf = np.zeros((128, NCONST_F), np.float32)
    wgr = w_gate.reshape(D, 5 * E)  # [d, t*E + e]
    cf[0:D, C_WG : C_WG + 5 * E] = wgr
    cf[D : 2 * D, C_WG : C_WG + 5 * E] = wgr
    cf[:, C_B1] = np.tile(conv1_b, 4)  # rows 64p+32q+oc -> conv1_b[oc]
    # f32 conv2 image: c2w[e, 32*ic+oc] = conv2_w[oc*8+e, ic, 0]
    c2 = np.zeros((E, NC2), np.float32)
    c2[:, C2_W : C2_W + OC * OC] = (
        conv2_w[:, :, 0].reshape(OC, E, OC).transpose(1, 2, 0).reshape(E, OC * OC)
    )
    return np.ascontiguousarray(cw), cf, np.ascontiguousarray(c2)


def _run(x, w_gate, conv1_w, conv1_b, conv2_w, conv2_b, **spmd_kwargs):
    x = np.asarray(x, np.float32)
    assert x.shape == (B, D, L), x.shape
    cw, cf, c2 = _prep_weights(w_gate, conv1_w, conv1_b, conv2_w, conv2_b)
    nc = _build()
    # xb[core, p, 64q+d, c] = x[4*core + 2q+p, d, c], bf16
    xr = x.reshape(NCORES, 2, 2, D, L)  # [core, q, p, d, c]
    xb = np.ascontiguousarray(xr.transpose(0, 2, 1, 3, 4)).reshape(
        NCORES, 2, 2 * D, L
    )
    xb16 = xb.astype(ml_dtypes.bfloat16)
    in_maps = []
    for i in range(NCORES):
        cfi = cf.copy()
        # gwin: cf[64q+d, C_GWIN + 2t + p] = x[2q+p, d, 4090+t] (fp32)
        win = xb[i, :, :, L - 6 : L - 1]  # [p, 64q+d, t]
        cfi[:, C_GWIN : C_GWIN + 10] = win.transpose(1, 2, 0).reshape(2 * D, 10)
        in_maps.append(
            {
                "x": np.ascontiguousarray(xb16[i]),
                "cw": cw,
                "cf": cfi,
                "c2": c2,
            }
        )
    res = bass_utils.run_bass_kernel_spmd(
        nc, in_maps, core_ids=list(range(NCORES)), **spmd_kwargs
    )
    y = np.concatenate([r["y"] for r in res.results], axis=0)
    return np.ascontiguousarray(y.astype(np.float32)), res


def kernel(x, w_gate, conv1_w, conv1_b, conv2_w, conv2_b):
    y, _ = _run(x, w_gate, conv1_w, conv1_b, conv2_w, conv2_b)
    return y


# revision 29
# speedup vs baseline: 1.2080x; 1.1391x over previous
"""MoE-routed conv kernel (Channel_Embedding ablation) for 8 trn2 NeuronCores.

Math (see reference):
  gates  = top2-renormalized softmax( x[:, :, -6:-1].reshape(B, D*5) @ w_gate )
  h      = tanh(conv1d(x, conv1_w, VALID) + conv1_b)            # [B, OC, L-2]
  out    = conv1d(h, conv2_w, 1x1) + conv2_b                    # [B, OC*E, L-2]
  y[b,oc,t] = sum_e gates[b,e] * out[b, oc*E+e, t]

Key algebraic fold: the expert combine commutes with the 1x1 conv, so per
batch element
  W_eff[b][oc, ic] = sum_e gates[b,e] * conv2_w[oc*E+e, ic, 0]
  b_eff[b][oc]     = sum_e gates[b,e] * conv2_b[oc*E+e]
  y[b] = W_eff[b] @ h[b] + b_eff[b]

Sharding: data-parallel over batch B=32 across 8 cores (4 each); weights
replicated.

Layout (per core, batches b = 2q + p, pair p, half q):
  - x is shipped bf16 as xb[p, 64q+d, c]; the rel-err budget (2e-2) is ~5x
    above the bf16 conv error (~4e-3). The 5-col gating window ships
    separately in fp32 so top-2 expert selection is exact.
  - conv1 weights are six zero-padded [128,128] bf16 blocks (tap k, pair p):
    rows 64q+d -> out cols 64p+32q+oc. Since matmul cost is per moving
    column (M is free), the zero half makes pair p's outputs land on PSUM
    partitions 64p..64p+31+32, so BOTH pairs accumulate into ONE PSUM bank
    (engines are lane-locked; this is what enables the merged layout).
  - per position tile [512]: 6 conv matmuls -> one PSUM [128,512] ->
    one tanh (+conv1 bias) -> hs bf16 [128,512] -> ONE combine matmul with
    block-diag Wc [128,128] bf16 (K=128, all 4 batches at once) -> one DVE
    drain (+b_eff) into a persistent y image [128, 4094] -> 3 large column-
    range stores (one per (p,q) slice), issued as halves complete.
  - Wc is built from W_eff = gates @ c2w via a small DRAM bounce with 4
    diagonal-block scatter reads; b_eff via one PE matmul + 4 tiny copies.

DMA budget: loads (4 x 512KB, 4KB rows) on Sync HWDGE; constants + Wc
scatter on Scalar HWDGE (second ring); bounce write on GpSimd SWDGE; the
3 y stores (8-16KB rows) on Sync. Each dma_start costs ~0.65us of issue
time on its sequencer, so counts are kept minimal and split across rings.
"""

from contextlib import ExitStack

import numpy as np
import ml_dtypes

import concourse.bacc as bacc
import concourse.mybir as mybir
import concourse.tile as tile
from concourse import bass_utils

B, D, L = 32, 64, 4096
E, TOPK, OC = 8, 2, 32
LP = L - 2  # 4094 valid conv outputs
NCORES = 8
NB = B // NCORES  # batch elements per core
TS = 512  # position tile (one PSUM bank of fp32)
NT = (LP + TS - 1) // TS

F32R = mybir.dt.float32r  # fp32 bits, fast PE mode (for tiny W_eff matmuls)
BF16 = mybir.dt.bfloat16

# f32 constants image [128, NCONST_F]:
#   wg   [*, 40] gating weights, duplicated in both partition halves,
#        col = 8*t + e
#   gwin [*, 20] gating window: rows 64q+d, col 4*t + b hold
#        x[b, d, 4090+t] for b in {2q, 2q+1}, zero for the other half's
#        batches — so one [64, 4] lhsT slice per (q, t) accumulates all
#        four batches' logits into a single [4, E] PSUM group
#   b1t  [*, 1]  conv1 bias by output row (64p+32q+oc -> conv1_b[oc])
C_WG = 0
C_GWIN = C_WG + 5 * E
C_B1 = C_GWIN + 4 * 5
NCONST_F = C_B1 + 1
# f32 conv2 image [8, NC2]: c2w[e, 32*ic+oc]
C2_W, NC2 = 0, OC * OC
# bf16 weight image [128, NWB]: conv1 blocks j = 3*p + k at cols 128j,
# then c2b_rep[e, 32*bb+oc] (rows 0:8) at cols NW1 (bf16 so the b_eff
# matmul can have M=128; fp32r pairs PE column groups and caps M at 64)
NW1 = 6 * 128
C2_B = NW1
NWB = NW1 + 4 * OC

_CACHE: dict = {}


def _softmax_top2(nc, sm, lg, f32, AX, OP, AF, q):
    """Per-half gating: lg [2, E] logits (PSUM) -> gates.T [32, 32] in SBUF.

    gates = (e >= m2) * e / (m1 + m2), e = exp(logits) - identical to
    softmax -> top2 -> vk/(sum vk + 1e-6) up to the 1e-6 term (negligible).
    """
    e_sb = sm.tile([2, E], f32, name=f"e_sb{q}")
    nc.scalar.activation(e_sb[:], lg[:], AF.Exp)
    m1 = sm.tile([2, 1], f32, name=f"m1_{q}")
    nc.vector.reduce_max(m1[:], e_sb[:], axis=AX.X)
    lt = sm.tile([2, E], f32, name=f"lt{q}")
    nc.vector.tensor_scalar(lt[:], e_sb[:], m1[:], None, op0=OP.is_lt)
    emsk = sm.tile([2, E], f32, name=f"emsk{q}")
    nc.vector.tensor_mul(emsk[:], lt[:], e_sb[:])  # e with the max zeroed
    m2 = sm.tile([2, 1], f32, name=f"m2_{q}")
    nc.vector.reduce_max(m2[:], emsk[:], axis=AX.X)
    den = sm.tile([2, 1], f32, name=f"den{q}")
    nc.vector.tensor_add(den[:], m1[:], m2[:])
    rcp = sm.tile([2, 1], f32, name=f"rcp{q}")
    nc.vector.reciprocal(rcp[:], den[:])
    ge = sm.tile([2, E], f32, name=f"ge{q}")
    nc.vector.tensor_scalar(ge[:], e_sb[:], m2[:], None, op0=OP.is_ge)
    gnum = sm.tile([2, E], f32, name=f"gnum{q}")
    nc.vector.tensor_mul(gnum[:], ge[:], e_sb[:])
    gpad = sm.tile([32, 32], f32, name=f"gpad{q}")
    nc.vector.memset(gpad[:], 0.0)
    nc.vector.tensor_scalar(gpad[0:2, 0:E], gnum[:], rcp[:], None, op0=OP.mult)
    gtr = sm.tile([32, 32], f32, name=f"gtr{q}")
    nc.vector.transpose(gtr[:], gpad[:])  # 32x32 block transpose
    return gtr  # gtr[0:E, 0:2] = gates.T for batches {2q, 2q+1}


def _emit(ctx, tc, nc, x_d, cw_d, cf_d, c2_d, y_d):
    f32 = mybir.dt.float32
    AF = mybir.ActivationFunctionType
    AX = mybir.AxisListType
    OP = mybir.AluOpType

    const = ctx.enter_context(tc.tile_pool(name="const", bufs=1))
    sm = ctx.enter_context(tc.tile_pool(name="sm", bufs=1))
    # all 8 hs tiles stay live so tanh (and thus conv PSUM recycling) never
    # stalls on combines, which wait for Wc (~13us) early in the run
    hsb = ctx.enter_context(tc.tile_pool(name="hsb", bufs=8))
    psum_h = ctx.enter_context(tc.tile_pool(name="ph", bufs=3, space="PSUM"))
    psum_y = ctx.enter_context(tc.tile_pool(name="py", bufs=3, space="PSUM"))
    psum_s = ctx.enter_context(tc.tile_pool(name="ps", bufs=2, space="PSUM"))
    dram = ctx.enter_context(tc.tile_pool(name="dram", bufs=1, space="DRAM"))

    # ---- ALL loads on the Sync HWDGE ring, in consumption order (FIFO per
    # ring): tiny cf (gating) first, cw (conv weights), then x chunks. A
    # second ring would round-robin against the big x transfers at packet
    # granularity and starve the small loads (measured: cw took 8us that
    # way). c2 (only needed by the W_eff matmul ~11us in) rides Scalar.
    cf = const.tile([128, NCONST_F], f32)
    nc.sync.dma_start(cf[:], cf_d.ap())
    cw = const.tile([128, NWB], BF16)
    nc.sync.dma_start(cw[:], cw_d.ap())
    xf = const.tile([2 * D, 2 * L], BF16)
    XSPLIT = 2048
    for p, a0, a1 in [(0, 0, XSPLIT), (1, 0, XSPLIT), (0, XSPLIT, L), (1, XSPLIT, L)]:
        nc.sync.dma_start(
            xf[:, L * p + a0 : L * p + a1],
            x_d.ap()[p, :, a0:a1],
        )
    c2 = const.tile([E, NC2], F32R)
    nc.scalar.dma_start(c2[:], c2_d.ap().bitcast(F32R))
    b1t = cf[:, C_B1 : C_B1 + 1]

    # ---- ACT table warmup (exp/tanh share one table set; load it early)
    warm = sm.tile([1, 8], f32)
    nc.vector.memset(warm[:], 0.0)
    warm2 = sm.tile([1, 8], f32)
    nc.scalar.activation(warm2[:], warm[:], AF.Exp)

    # ---- PE warmup: dummy matmuls (no data deps) keep the PE busy through
    # the load window so the clock ramp (3us continuous -> 2.4GHz) finishes
    # before the first real conv matmul.
    wsrc = sm.tile([128, 128], BF16)
    nc.vector.memset(wsrc[:].bitcast(f32), 0.0)
    wup = psum_s.tile([128, TS], f32, tag="s")
    for _ in range(10):
        nc.tensor.matmul(wup[:, 0:128], wsrc[:], wsrc[:], start=True, stop=True)

    # ---- gating: one [NB, E] PSUM accumulation over (q, t);
    # logits[b, e] = sum_{d,t} gwin[64q+d, 4t+b] * wg[64q+d, 8t+e]
    # (gwin rows 64q+d are zero for the other half's batch columns)
    gtrs = []
    for q in range(2):
        lgq = psum_s.tile([2, E], f32, tag="s", name=f"lg{q}")
        for t in range(5):
            nc.tensor.matmul(
                lgq[:],
                cf[D * q : D * q + D, C_GWIN + 4 * t + 2 * q : C_GWIN + 4 * t + 2 * q + 2],
                cf[D * q : D * q + D, C_WG + E * t : C_WG + E * t + E],
                start=(t == 0),
                stop=(t == 4),
            )
        gtrs.append(_softmax_top2(nc, sm, lgq, f32, AX, OP, AF, q))
    gT = sm.tile([E, NB], F32R)  # col b = 2q + p
    gTb = sm.tile([E, NB], BF16)
    for q in range(2):
        nc.vector.tensor_copy(gT[:, 2 * q : 2 * q + 2], gtrs[q][0:E, 0:2])
        nc.vector.tensor_copy(gTb[:, 2 * q : 2 * q + 2], gtrs[q][0:E, 0:2])
    # column view in bb = 2p + q order (matches output row blocks 32bb+oc)
    gTbb = gTb[:].rearrange("e (q p) -> e p q", q=2)  # 3D: col order bb = 2p+q

    # ---- b_eff[128,1]: rows 32bb+oc. One PE matmul gives
    # bp[32bb+oc, bb'] = sum_e c2b_rep[e, 32bb+oc] * gates[b(bb'), e];
    # the diagonal blocks are extracted with 4 tiny copies.
    bp = psum_s.tile([OC * NB, NB], f32, tag="s")
    nc.tensor.matmul(
        bp[:],
        cw[0:E, C2_B : C2_B + 4 * OC],
        gTbb,
        start=True,
        stop=True,
    )
    beff = const.tile([OC * NB, 1], f32)
    for bb in range(NB):
        nc.vector.tensor_copy(
            beff[OC * bb : OC * bb + OC, :], bp[OC * bb : OC * bb + OC, bb : bb + 1]
        )

    # ---- W_eff[b] = gates[b] @ c2w -> weffb[b, 32*ic+oc] (bf16), then a
    # DRAM bounce scatters the four [32,32] diagonal blocks into Wc.
    # Casts split DVE/ACT so they run in parallel; bounce DMAs on Sync
    # (issue right after the x loads) so Scalar's tanh stream is never
    # blocked behind a waiting DMA issue.
    weffb = sm.tile([NB, OC * OC], BF16)
    for hh in range(2):
        wp = psum_s.tile([NB, 512], f32, tag="s", name=f"wp{hh}")
        nc.tensor.matmul(
            wp[:],
            gT[:],
            c2[:, C2_W + 512 * hh : C2_W + 512 * (hh + 1)],
            start=True,
            stop=True,
        )
        nc.vector.tensor_copy(weffb[:, 512 * hh : 512 * (hh + 1)], wp[:])
    wscr = dram.tile([NB, OC * OC], BF16)
    nc.sync.dma_start(wscr[:], weffb[:])
    Wc = const.tile([128, 128], BF16)
    nc.vector.memset(Wc[:].bitcast(f32), 0.0)
    for bb in range(NB):
        p, q = bb // 2, bb % 2
        b = 2 * q + p
        nc.sync.dma_start(
            Wc[OC * bb : OC * bb + OC, OC * bb : OC * bb + OC],
            wscr[b : b + 1, :].rearrange("one (ic oc) -> (one ic) oc", ic=OC),
        )

    # persistent output image: yb[64p+32q+oc, c] = y[2q+p, oc, c]
    yb = const.tile([128, LP], f32)

    # ---- main loop over position tiles, software-pipelined one stage so
    # the first combine (needs Wc ~5us in) trails the first conv.
    stage = []  # (hp, hs, c0, n) pending tanh/combine/drain

    def finish(hp, hs, c0, n):
        nc.scalar.activation(hs[:, 0:n], hp[:, 0:n], AF.Tanh, bias=b1t, scale=1.0)
        op_ = psum_y.tile([128, TS], f32, tag="op")
        nc.tensor.matmul(op_[:, 0:n], Wc[:], hs[:, 0:n], start=True, stop=True)
        nc.vector.tensor_scalar(
            yb[:, c0 : c0 + n], op_[:, 0:n], beff[:], None, op0=OP.add
        )

    def store(a0, a1):
        # one DMA per (p, q) slice, split across both HWDGE rings so the
        # ~0.65us-per-DMA issue cost is paid in parallel
        for p in range(2):
            eng = nc.sync
            for q in range(2):
                eng.dma_start(
                    y_d.ap()[2 * q + p, :, a0:a1],
                    yb[64 * p + 32 * q : 64 * p + 32 * q + 32, a0:a1],
                )

    for t in range(NT):
        c0 = t * TS
        n = min(TS, LP - c0)
        hp = psum_h.tile([128, TS], f32, tag="hp")
        for p in range(2):
            for k in range(3):
                nc.tensor.matmul(
                    hp[:, 0:n],
                    cw[:, 128 * (3 * p + k) : 128 * (3 * p + k) + 128],
                    xf[:, L * p + c0 + k : L * p + c0 + k + n],
                    start=(p == 0 and k == 0),
                    stop=(p == 1 and k == 2),
                )
        hs = hsb.tile([128, TS], BF16, tag="hs")
        stage.append((hp, hs, c0, n))
        if len(stage) > 1:
            finish(*stage.pop(0))
        if t == 4:
            store(0, 4 * TS)  # tiles 0-3 drained by now (pipeline lag 1)
        elif t == 7:
            store(4 * TS, 7 * TS)
    finish(*stage.pop(0))
    store(7 * TS, LP)


def _build():
    if "nc" in _CACHE:
        return _CACHE["nc"]
    nc = bacc.Bacc(
        "TRN2",
        target_bir_lowering=False,
        debug=False,
        num_devices=NCORES,
        detect_race_conditions=False,
    )
    f32 = mybir.dt.float32
    x_d = nc.dram_tensor("x", [2, 2 * D, L], BF16, kind="ExternalInput")
    cw_d = nc.dram_tensor("cw", [128, NWB], BF16, kind="ExternalInput")
    cf_d = nc.dram_tensor("cf", [128, NCONST_F], f32, kind="ExternalInput")
    c2_d = nc.dram_tensor("c2", [E, NC2], f32, kind="ExternalInput")
    y_d = nc.dram_tensor("y", [NB, OC, LP], f32, kind="ExternalOutput")

    with tile.TileContext(nc) as tc:
        with ExitStack() as ctx:
            _emit(ctx, tc, nc, x_d, cw_d, cf_d, c2_d, y_d)
    nc.compile()
    _CACHE["nc"] = nc
    return nc


def _prep_weights(w_gate, conv1_w, conv1_b, conv2_w, conv2_b):
    w_gate = np.asarray(w_gate, np.float32)
    conv1_w = np.asarray(conv1_w, np.float32)
    conv1_b = np.asarray(conv1_b, np.float32)
    conv2_w = np.asarray(conv2_w, np.float32)
    conv2_b = np.asarray(conv2_b, np.float32)
    # bf16 conv1 blocks: cw[:, 128j:128j+128], j = 3p+k:
    #   rows 64q+d -> cols 64p+32q+oc = conv1_w[oc, d, k]
    cw = np.zeros((128, NWB), np.float32)
    wkt = conv1_w.transpose(1, 2, 0)  # [d, k, oc]
    for p in range(2):
        for k in range(3):
            j = 3 * p + k
            for q in range(2):
                cw[
                    D * q : D * q + D,
                    128 * j + 64 * p + 32 * q : 128 * j + 64 * p + 32 * q + OC,
                ] = wkt[:, k, :]
    # c2b_rep[e, 32*bb+oc] = conv2_b[oc*8+e] for every bb
    cw[0:E, C2_B : C2_B + 4 * OC] = np.tile(conv2_b.reshape(OC, E).T, (1, 4))
    cw = cw.astype(ml_dtypes.bfloat16)
    # f32 image (gwin filled per-core later)
    cf = np.zeros((128, NCONST_F), np.float32)
    wgr = w_gate.reshape(D, 5 * E)  # [d, t*E + e]
    cf[0:D, C_WG : C_WG + 5 * E] = wgr
    cf[D : 2 * D, C_WG : C_WG + 5 * E] = wgr
    cf[:, C_B1] = np.tile(conv1_b, 4)  # rows 64p+32q+oc -> conv1_b[oc]
    # f32 conv2 image: c2w[e, 32*ic+oc] = conv2_w[oc*8+e, ic, 0]
    c2 = np.zeros((E, NC2), np.float32)
    c2[:, C2_W : C2_W + OC * OC] = (
        conv2_w[:, :, 0].reshape(OC, E, OC).transpose(1, 2, 0).reshape(E, OC * OC)
    )
    return np.ascontiguousarray(cw), cf, np.ascontiguousarray(c2)


def _run(x, w_gate, conv1_w, conv1_b, conv2_w, conv2_b, **spmd_kwargs):
    x = np.asarray(x, np.float32)
    assert x.shape == (B, D, L), x.shape
    cw, cf, c2 = _prep_weights(w_gate, conv1_w, conv1_b, conv2_w, conv2_b)
    nc = _build()
    # xb[core, p, 64q+d, c] = x[4*core + 2q+p, d, c], bf16
    xr = x.reshape(NCORES, 2, 2, D, L)  # [core, q, p, d, c]
    xb = np.ascontiguousarray(xr.transpose(0, 2, 1, 3, 4)).reshape(
        NCORES, 2, 2 * D, L
    )
    xb16 = xb.astype(ml_dtypes.bfloat16)
    in_maps = []
    for i in range(NCORES):
        cfi = cf.copy()
        # gwin: cf[64q+d, C_GWIN + 4t + b] = x[b, d, 4090+t] for b in
        # {2q, 2q+1}, zero for the other half's batches
        win = xb[i, :, :, L - 6 : L - 1]  # [p, 64q+d, t]
        garr = np.zeros((2, D, 5, NB), np.float32)
        for q in range(2):
            for p in range(2):
                garr[q, :, :, 2 * q + p] = win[p, D * q : D * q + D, :]
        cfi[:, C_GWIN : C_GWIN + 4 * 5] = garr.reshape(2 * D, 4 * 5)
        in_maps.append(
            {
                "x": np.ascontiguousarray(xb16[i]),
                "cw": cw,
                "cf": cfi,
                "c2": c2,
            }
        )
    res = bass_utils.run_bass_kernel_spmd(
        nc, in_maps, core_ids=list(range(NCORES)), **spmd_kwargs
    )
    y = np.concatenate([r["y"] for r in res.results], axis=0)
    return np.ascontiguousarray(y.astype(np.float32)), res


def kernel(x, w_gate, conv1_w, conv1_b, conv2_w, conv2_b):
    y, _ = _run(x, w_gate, conv1_w, conv1_b, conv2_w, conv2_b)
    return y
